# revision 17
# baseline (speedup 1.0000x reference)
"""S4ND Darcy-flow Bass kernel v2: builder + host-side preparation.

Design (per core = one batch element, batch-parallel over 4 cores, cores
4..7 duplicate work and are ignored at gather time):

  state h_sb: SBUF bf16 [128p=h, (w, d)], d innermost.
  Encoder precomputed on host (h0 DMA'd straight into h_sb).
  Per layer:
    phase 1 (conv), channels in groups of 4:
      MM1 x4:  ps1[:, j*128] = U_d^T @ ThT_d        (lhsT = U_d strided slice)
      copy:    ps1 -> At4 bf16 (DVE/ACT alternating, [128,512])
      sident:  DVE tensor_scalar builds D_d * I from identity
      MM2 x4:  ps2[:, j*128] = At^T @ TwT_d  (+ D_d*I^T @ U_d accumulated)
      gelu:    one ACT op [128,512] -> G[p, ch, w] (dense write)
      stage B: per 8 w0: 8 PE transposes into one PSUM bank [128,1024] bf16,
               one DVE/ACT copy out to Xt (channel-major).
    phase 2 (per w0): GEMM psW = Xt0@W0 + Xt1@W1; ACT sigmoid; DVE glu
      (+ssum accum via sum(glu)=sum(p), valid post-LN layers); DVE residual
      add in place into h_sb; DVE sumsq accum; batched stats; normalize
      in place (ACT Identity with per-partition scale/bias, DVE share).
  Decoder: DVE stt dot-products per w slice -> out (h, w) f32.

Host precomputes (numpy, float64): S4D kernels kh/kw, transposed Toeplitz
matrices ThT/TwT packed 4 channels per tile row for dense DMA.
"""

import numpy as np
import ml_dtypes

import concourse.bacc as bacc
import concourse.mybir as mybir
import concourse.tile as tile

bf16 = ml_dtypes.bfloat16
AF = mybir.ActivationFunctionType
OP = mybir.AluOpType
F32 = mybir.dt.float32
BF = mybir.dt.bfloat16

H = 128
W = 128

# normalize assignment pattern: of every 4 w0, this many normalize on DVE
# (via stt with broadcast in1); the rest on ACT (Identity w/ scale+bias).
NORM_DVE_FRACTION = 2


def host_prep(inputs, n_layers=None, d_model=None):
    """Compute device-side constant tensors from the full model inputs."""
    log_dt = np.asarray(inputs["log_dt"], np.float64)     # (L,2,d)
    logA_re = np.asarray(inputs["logA_re"], np.float64)   # (L,2,d,N)
    A_im = np.asarray(inputs["A_im"], np.float64)
    C_re = np.asarray(inputs["C_re"], np.float64)
    C_im = np.asarray(inputs["C_im"], np.float64)
    Dskip = np.asarray(inputs["Dskip"], np.float64)       # (L,d)
    W_out = np.asarray(inputs["W_out"], np.float64)       # (L,d,2d)
    b_out = np.asarray(inputs["b_out"], np.float64)       # (L,2d)
    ln_w = np.asarray(inputs["ln_w"], np.float64)         # (L,d)
    ln_b = np.asarray(inputs["ln_b"], np.float64)
    W_enc = np.asarray(inputs["W_enc"], np.float64)       # (2,d)
    b_enc = np.asarray(inputs["b_enc"], np.float64)       # (d,)
    W_dec = np.asarray(inputs["W_dec"], np.float64)       # (d,1)
    b_dec = np.asarray(inputs["b_dec"], np.float64)       # (1,)
    x = np.asarray(inputs["x"], np.float64)               # (B,H,W,1)
    grid = np.asarray(inputs["grid"], np.float64)

    L = log_dt.shape[0] if n_layers is None else n_layers
    D = log_dt.shape[2] if d_model is None else d_model
    log_dt = log_dt[:L, :, :D]
    logA_re = logA_re[:L, :, :D]
    A_im = A_im[:L, :, :D]
    C_re = C_re[:L, :, :D]
    C_im = C_im[:L, :, :D]
    Dskip = Dskip[:L, :D]
    d_full = W_out.shape[1]
    Wa = W_out[:L, :D, :D]
    Wg = W_out[:L, :D, d_full:d_full + D]
    W_out2 = np.concatenate([Wa, Wg], axis=2)             # (L, D, 2D)
    b_out2 = np.concatenate([b_out[:L, :D], b_out[:L, d_full:d_full + D]], axis=1)
    ln_w = ln_w[:L, :D]
    ln_b = ln_b[:L, :D]
    W_enc = W_enc[:, :D]
    b_enc = b_enc[:D]
    W_dec = W_dec[:D]

    # ---- S4D kernels ----
    dt = np.exp(log_dt)[..., None]                        # (L,2,D,1)
    A = -np.exp(logA_re) + 1j * A_im                      # (L,2,D,N)
    C = C_re + 1j * C_im
    dtA = dt * A
    CB = C * (np.exp(dtA) - 1.0) / A
    t = np.arange(H, dtype=np.float64)
    pows = np.exp(dtA[..., None] * t)                     # (L,2,D,N,H)
    K = 2.0 * np.real(np.einsum("lxdn,lxdnt->lxdt", CB, pows))  # (L,2,D,H)
    kh = K[:, 0]                                          # (L,D,H)
    kw = K[:, 1]                                          # (L,D,W)

    # transposed lower-triangular Toeplitz: ThT[l,d,i,p] = kh[l,d,p-i], p>=i
    idx = np.arange(H)[None, :] - np.arange(H)[:, None]   # (i,p) = p-i
    mask = idx >= 0
    idxc = np.clip(idx, 0, H - 1)
    ThT = np.where(mask, kh[:, :, idxc], 0.0)             # (L,D,128,128)
    TwT = np.where(mask, kw[:, :, idxc], 0.0)

    def pack4(T):
        # (L, D, 128, 128) -> (L, D//4, 128, 512): 4 channels side by side
        Lc, Dc = T.shape[0], T.shape[1]
        return np.ascontiguousarray(
            T.reshape(Lc, Dc // 4, 4, 128, 128)
             .transpose(0, 1, 3, 2, 4)
             .reshape(Lc, Dc // 4, 128, 512)
             .astype(np.float32).astype(bf16))

    flags = dict(
        use_ln_affine=not (np.all(ln_w == 1.0) and np.all(ln_b == 0.0)),
        use_b_out=not np.all(b_out2 == 0.0),
        n_layers=L,
        d_model=D,
        b_dec=float(b_dec[0]),
    )

    flags["wsum"] = float(np.sum(W_dec))

    NK = max(1, D // 128)
    common = dict(
        tht4=pack4(ThT),                                  # (L,D/4,128,512)
        twt4=pack4(TwT),
        drep=np.tile(Dskip.astype(np.float32)[:, None, :], (1, 128, 1)),  # (L,128,D)
        wdec_rep=np.tile(W_dec.astype(np.float32).reshape(1, D), (128, 1)).astype(bf16),
        ident=np.eye(128, dtype=np.float32).astype(bf16),
        wout=np.ascontiguousarray(
            W_out2.reshape(L, NK, min(D, 128), 2 * D).astype(np.float32).astype(bf16)),
    )
    if flags["use_ln_affine"]:
        common["lnw_rep"] = np.tile(ln_w.astype(np.float32)[:, None, :], (1, 128, 1)).astype(bf16)
        common["lnb_rep"] = np.tile(ln_b.astype(np.float32)[:, None, :], (1, 128, 1)).astype(bf16)
    if flags["use_b_out"]:
        common["bout_rep"] = np.tile(b_out2.astype(np.float32)[:, None, :], (1, 128, 1))

    # host-side encoder: h0[b, h, w, d] = x*We0 + grid*We1 + b_enc
    xg = np.stack([x[..., 0], grid[..., 0]], axis=-1)     # (B,H,W,2)
    h0 = xg @ W_enc + b_enc                               # (B,H,W,D) f64
    per_batch = []
    for b in range(x.shape[0]):
        per_batch.append(dict(h0=np.ascontiguousarray(
            h0[b].astype(np.float32).astype(bf16))))      # (H, W, D) = (128, w, d)
    return common, per_batch, flags


def build_program(flags, num_devices=8, gelu_fn=None):
    """Emit the bass program."""
    L = flags["n_layers"]
    D = flags["d_model"]
    NG = D // 4            # channel groups of 4
    NK = max(1, D // 128)  # K tiles in W_out GEMM
    DH = D // 2
    assert D % 8 == 0

    if gelu_fn is None:
        gelu_fn = AF.Gelu_apprx_tanh
    nc = bacc.Bacc("TRN2", target_bir_lowering=False, debug=False,
                   num_devices=num_devices)

    def din(name, shape, dt):
        return nc.dram_tensor(name, shape, dt, kind="ExternalInput").ap()

    h0 = din("h0", [128, W, D], BF)
    tht4 = din("tht4", [L, NG, 128, 512], BF)
    twt4 = din("twt4", [L, NG, 128, 512], BF)
    drep = din("drep", [L, 128, D], F32)
    wout = din("wout", [L, NK, min(D, 128), 2 * D], BF)
    wdec_rep = din("wdec_rep", [128, D], BF)
    ident = din("ident", [128, 128], BF)
    if flags["use_ln_affine"]:
        lnw_rep = din("lnw_rep", [L, 128, D], BF)
        lnb_rep = din("lnb_rep", [L, 128, D], BF)
    if flags["use_b_out"]:
        bout_rep = din("bout_rep", [L, 128, 2 * D], F32)
    out = nc.dram_tensor("out", [H, W], F32, kind="ExternalOutput").ap()

    from contextlib import ExitStack
    with tile.TileContext(nc) as tc, ExitStack() as ctx:
        state = ctx.enter_context(tc.tile_pool(name="state", bufs=1))
        consts = ctx.enter_context(tc.tile_pool(name="consts", bufs=1))
        lring = ctx.enter_context(tc.tile_pool(name="lring", bufs=2))
        wring = ctx.enter_context(tc.tile_pool(name="wring", bufs=8))
        atring = ctx.enter_context(tc.tile_pool(name="atring", bufs=3))
        cring = ctx.enter_context(tc.tile_pool(name="cring", bufs=6))
        stats = ctx.enter_context(tc.tile_pool(name="stats", bufs=1))
        psA = ctx.enter_context(tc.tile_pool(name="psA", bufs=2, space="PSUM"))
        psB = ctx.enter_context(tc.tile_pool(name="psB", bufs=2, space="PSUM"))
        psT = ctx.enter_context(tc.tile_pool(name="psT", bufs=2, space="PSUM"))
        psC = ctx.enter_context(tc.tile_pool(name="psC", bufs=2, space="PSUM"))

        h_sb = state.tile([128, W, D], BF, tag="h")
        G = state.tile([128, DH, W], BF, tag="G")   # [p, ch-in-half, w]
        Xts = [state.tile([128, H * W], BF, tag=f"xt{k}", name=f"xt{k}")
               for k in range(NK)]

        ident_t = consts.tile([128, 128], BF)
        nc.sync.dma_start(ident_t[:], ident[:])
        wdec_t = consts.tile([128, D], BF)
        nc.sync.dma_start(wdec_t[:], wdec_rep[:])
        eps_t = consts.tile([128, 1], F32)
        nc.vector.memset(eps_t[:], 1e-5)

        ssum = stats.tile([128, W], F32, tag="ssum")
        sqs = stats.tile([128, W], F32, tag="sqs")
        mu = stats.tile([128, W], F32, tag="mu")
        var = stats.tile([128, W], F32, tag="var")
        std = stats.tile([128, W], F32, tag="std")
        rr = stats.tile([128, W], F32, tag="rr")
        nmr = stats.tile([128, W], F32, tag="nmr")
        dec_sb = stats.tile([128, W], F32, tag="dec")

        # load encoder output straight into the state
        nc.sync.dma_start(h_sb[:], h0[:])

        for l in range(L):
            wout_ts = []
            for k in range(NK):
                wt = lring.tile([min(D, 128), 2 * D], BF, tag="woutw")
                nc.sync.dma_start(wt[:], wout[l, k])
                wout_ts.append(wt)
            drep_t = lring.tile([128, D], F32, tag="drep")
            nc.sync.dma_start(drep_t[:], drep[l])
            if flags["use_ln_affine"]:
                lnw_t = lring.tile([128, D], BF, tag="lnw")
                nc.sync.dma_start(lnw_t[:], lnw_rep[l])
                lnb_t = lring.tile([128, D], BF, tag="lnb")
                nc.sync.dma_start(lnb_t[:], lnb_rep[l])
            if flags["use_b_out"]:
                bout_t = lring.tile([128, 2 * D], F32, tag="bout")
                nc.sync.dma_start(bout_t[:], bout_rep[l])

            # ---------------- phase 1: conv + transpose ----------------
            for hf in range(2):
                ngh = NG // 2
                for g in range(ngh):
                    gd = hf * ngh + g
                    d0 = gd * 4
                    tht_t = wring.tile([128, 512], BF, tag="tht")
                    nc.sync.dma_start(tht_t[:], tht4[l, gd])
                    twt_t = wring.tile([128, 512], BF, tag="twt")
                    nc.sync.dma_start(twt_t[:], twt4[l, gd])
                    ps1 = psA.tile([128, 512], F32, tag="psA")
                    for j in range(4):
                        nc.tensor.matmul(ps1[:, j * 128:(j + 1) * 128],
                                         h_sb[:, :, d0 + j], tht_t[:, j * 128:(j + 1) * 128],
                                         start=True, stop=True)
                    At4 = atring.tile([128, 512], BF, tag="at")
                    if g % 2 == 0:
                        nc.vector.tensor_copy(At4[:], ps1[:])
                    else:
                        nc.scalar.copy(At4[:], ps1[:])
                    ps2 = psB.tile([128, 512], F32, tag="psB")
                    for j in range(4):
                        nc.tensor.matmul(ps2[:, j * 128:(j + 1) * 128],
                                         At4[:, j * 128:(j + 1) * 128],
                                         twt_t[:, j * 128:(j + 1) * 128],
                                         start=True, stop=True)
                    # D-skip on DVE (idle in phase 1): ps2 += D_d * u_d
                    for j in range(4):
                        d = d0 + j
                        nc.vector.scalar_tensor_tensor(
                            out=ps2[:, j * 128:(j + 1) * 128],
                            in0=h_sb[:, :, d], scalar=drep_t[:, d:d + 1],
                            in1=ps2[:, j * 128:(j + 1) * 128],
                            op0=OP.mult, op1=OP.add)
                    # gelu -> G (dense write, [p, (c w)])
                    gg = g * 4
                    gout = G[:, gg:gg + 4, :].rearrange("p c w -> p (c w)")
                    nc.scalar.activation(gout, ps2[:], gelu_fn)

                # ---- stage B: transpose this half into Xt ----
                for wb in range(0, W, 8):
                    pst = psT.tile([128, 1024], BF, tag="psT")
                    for t in range(8):
                        nc.tensor.transpose(pst[:, t * 128:(t + 1) * 128],
                                            G[:, :, wb + t], ident_t[:])
                    dst = Xts[hf][:, wb * 128:(wb + 8) * 128]
                    if (wb // 8) % 2 == 0:
                        nc.vector.tensor_copy(dst, pst[:])
                    else:
                        nc.scalar.copy(dst, pst[:])

            # ---------------- phase 2: GEMM + GLU + residual + LN ----------
            # last layer: LN output feeds only the decoder; fold both:
            # out = rr*(sum(p*wdec) - mu*sum(wdec)) + b_dec
            fold_dec = (l == L - 1) and not flags["use_ln_affine"]
            SB = 64  # stats batch
            for w0 in range(W):
                psW = psC.tile([128, 2 * D], F32, tag="psC")
                for k in range(NK):
                    nc.tensor.matmul(psW[:], Xts[k][:, w0 * 128:(w0 + 1) * 128],
                                     wout_ts[k][:], start=(k == 0),
                                     stop=(k == NK - 1))
                if flags["use_b_out"]:
                    nc.vector.tensor_tensor(psW[:], psW[:], bout_t[:], op=OP.add)
                sig = cring.tile([128, D], BF, tag="sig")
                nc.scalar.activation(sig[:], psW[:, D:2 * D], AF.Sigmoid)
                glu = cring.tile([128, D], BF, tag="glu")
                h_slot = h_sb[:, w0, :]
                if l == 0:
                    # encoder h is not zero-mean: accumulate sum on residual
                    nc.vector.scalar_tensor_tensor(
                        out=glu[:], in0=psW[:, 0:D], scalar=1.0, in1=sig[:],
                        op0=OP.mult, op1=OP.mult)
                    nc.vector.scalar_tensor_tensor(
                        out=h_slot, in0=glu[:], scalar=1.0, in1=h_slot,
                        op0=OP.mult, op1=OP.add, accum_out=ssum[:, w0:w0 + 1])
                else:
                    # post-LN h sums to zero over d: sum(p) == sum(glu)
                    nc.vector.scalar_tensor_tensor(
                        out=glu[:], in0=psW[:, 0:D], scalar=1.0, in1=sig[:],
                        op0=OP.mult, op1=OP.mult, accum_out=ssum[:, w0:w0 + 1])
                    nc.vector.tensor_tensor(h_slot, glu[:], h_slot, op=OP.add)
                scr = cring.tile([128, D], BF, tag="scr")
                nc.vector.scalar_tensor_tensor(
                    out=scr[:], in0=h_slot, scalar=1.0, in1=h_slot,
                    op0=OP.mult, op1=OP.mult, accum_out=sqs[:, w0:w0 + 1])
                if fold_dec:
                    scr2 = cring.tile([128, D], BF, tag="scr2")
                    nc.vector.scalar_tensor_tensor(
                        out=scr2[:], in0=h_slot, scalar=1.0, in1=wdec_t[:],
                        op0=OP.mult, op1=OP.mult,
                        accum_out=dec_sb[:, w0:w0 + 1])
                if w0 % SB == SB - 1:
                    b0 = w0 - SB + 1
                    blk = slice(b0, w0 + 1)
                    nc.vector.tensor_scalar(out=mu[:, blk], in0=ssum[:, blk],
                                            scalar1=1.0 / D, scalar2=None,
                                            op0=OP.mult)
                    nc.vector.tensor_tensor(var[:, blk], mu[:, blk], mu[:, blk],
                                            op=OP.mult)
                    nc.vector.scalar_tensor_tensor(
                        out=var[:, blk], in0=sqs[:, blk], scalar=1.0 / D,
                        in1=var[:, blk], op0=OP.mult, op1=OP.subtract)
                    nc.scalar.activation(std[:, blk], var[:, blk], AF.Sqrt,
                                         bias=eps_t[:, 0:1])
                    nc.vector.reciprocal(rr[:, blk], std[:, blk])
                    if fold_dec:
                        # dec = (dec_raw - mu*wsum) * rr
                        nc.vector.scalar_tensor_tensor(
                            out=dec_sb[:, blk], in0=mu[:, blk],
                            scalar=-float(flags["wsum"]),
                            in1=dec_sb[:, blk], op0=OP.mult, op1=OP.add)
                        nc.vector.tensor_tensor(dec_sb[:, blk], dec_sb[:, blk],
                                                rr[:, blk], op=OP.mult)
                        continue
                    nc.vector.scalar_tensor_tensor(
                        out=nmr[:, blk], in0=mu[:, blk], scalar=-1.0,
                        in1=rr[:, blk], op0=OP.mult, op1=OP.mult)
                    for wv in range(b0, w0 + 1):
                        hv = h_sb[:, wv, :]
                        # first block: ACT only (DVE still busy with the
                        # remaining w0); tail block: split DVE/ACT to halve
                        # the serial tail before the next layer's conv.
                        if wv >= W - SB and wv % 2 == 0:
                            nc.vector.scalar_tensor_tensor(
                                out=hv, in0=hv, scalar=rr[:, wv:wv + 1],
                                op0=OP.mult,
                                in1=nmr[:, wv:wv + 1].broadcast_to((128, D)),
                                op1=OP.add)
                        else:
                            nc.scalar.activation(
                                hv, hv, AF.Identity,
                                bias=nmr[:, wv:wv + 1], scale=rr[:, wv:wv + 1])
                        if flags["use_ln_affine"]:
                            nc.vector.tensor_tensor(hv, hv, lnw_t[:], op=OP.mult)
                            nc.vector.tensor_tensor(hv, hv, lnb_t[:], op=OP.add)

        # ---------------- decoder (only if not folded into last layer) ------
        if flags["use_ln_affine"]:
            for w0 in range(W):
                scr = cring.tile([128, D], BF, tag="scr")
                nc.vector.scalar_tensor_tensor(
                    out=scr[:], in0=h_sb[:, w0, :], scalar=1.0, in1=wdec_t[:],
                    op0=OP.mult, op1=OP.mult, accum_out=dec_sb[:, w0:w0 + 1])
        if flags["b_dec"] != 0.0:
            nc.vector.tensor_scalar(out=dec_sb[:], in0=dec_sb[:],
                                    scalar1=float(flags["b_dec"]), scalar2=None,
                                    op0=OP.add)
        nc.sync.dma_start(out[:], dec_sb[:])

    nc.compile()
    return nc


# ---------------------------------------------------------------------------
# Self-contained entry point: full inputs in, full output out.
# ---------------------------------------------------------------------------

_PROGRAM_CACHE = {}


def _get_program(flags):
    key = (flags["n_layers"], flags["d_model"], flags["use_ln_affine"],
           flags["use_b_out"], flags["b_dec"], flags.get("wsum"))
    if key not in _PROGRAM_CACHE:
        _PROGRAM_CACHE[key] = build_program(flags, num_devices=8)
    return _PROGRAM_CACHE[key]


def kernel(**inputs):
    import os
    from concourse.bass_utils import run_bass_kernel_spmd

    common, per_batch, flags = host_prep(inputs)
    nc = _get_program(flags)

    B = len(per_batch)
    in_maps = []
    for c in range(8):
        m = dict(common)
        m.update(per_batch[c % B])
        in_maps.append(m)

    trace = bool(os.environ.get("S4ND_TRACE"))
    res = run_bass_kernel_spmd(nc, in_maps, core_ids=list(range(8)), trace=trace)
    if trace and res.exec_time_ns is not None:
        print(f"HW exec time: {res.exec_time_ns} ns")
        kernel.last_exec_time_ns = res.exec_time_ns
        kernel.last_results = res

    out = np.stack([res.results[b]["out"] for b in range(B)], axis=0)[..., None]
    return out.astype(np.float32)


# revision 22
# speedup vs baseline: 1.0287x; 1.0287x over previous
"""S4ND Darcy-flow Bass kernel v2: builder + host-side preparation.

Design (per core = one batch element, batch-parallel over 4 cores, cores
4..7 duplicate work and are ignored at gather time):

  state h_sb: SBUF bf16 [128p=h, (w, d)], d innermost.
  Encoder precomputed on host (h0 DMA'd straight into h_sb).
  Per layer:
    phase 1 (conv), channels in groups of 4:
      MM1 x4:  ps1[:, j*128] = U_d^T @ ThT_d        (lhsT = U_d strided slice)
      copy:    ps1 -> At4 bf16 (DVE/ACT alternating, [128,512])
      sident:  DVE tensor_scalar builds D_d * I from identity
      MM2 x4:  ps2[:, j*128] = At^T @ TwT_d  (+ D_d*I^T @ U_d accumulated)
      gelu:    one ACT op [128,512] -> G[p, ch, w] (dense write)
      stage B: per 8 w0: 8 PE transposes into one PSUM bank [128,1024] bf16,
               one DVE/ACT copy out to Xt (channel-major).
    phase 2 (per w0): GEMM psW = Xt0@W0 + Xt1@W1; ACT sigmoid; DVE glu
      (+ssum accum via sum(glu)=sum(p), valid post-LN layers); DVE residual
      add in place into h_sb; DVE sumsq accum; batched stats; normalize
      in place (ACT Identity with per-partition scale/bias, DVE share).
  Decoder: DVE stt dot-products per w slice -> out (h, w) f32.

Host precomputes (numpy, float64): S4D kernels kh/kw, transposed Toeplitz
matrices ThT/TwT packed 4 channels per tile row for dense DMA.
"""

import numpy as np
import ml_dtypes

import concourse.bacc as bacc
import concourse.mybir as mybir
import concourse.tile as tile

bf16 = ml_dtypes.bfloat16
AF = mybir.ActivationFunctionType
OP = mybir.AluOpType
F32 = mybir.dt.float32
BF = mybir.dt.bfloat16

H = 128
W = 128

# normalize assignment pattern: of every 4 w0, this many normalize on DVE
# (via stt with broadcast in1); the rest on ACT (Identity w/ scale+bias).
NORM_DVE_FRACTION = 2


def host_prep(inputs, n_layers=None, d_model=None):
    """Compute device-side constant tensors from the full model inputs."""
    log_dt = np.asarray(inputs["log_dt"], np.float64)     # (L,2,d)
    logA_re = np.asarray(inputs["logA_re"], np.float64)   # (L,2,d,N)
    A_im = np.asarray(inputs["A_im"], np.float64)
    C_re = np.asarray(inputs["C_re"], np.float64)
    C_im = np.asarray(inputs["C_im"], np.float64)
    Dskip = np.asarray(inputs["Dskip"], np.float64)       # (L,d)
    W_out = np.asarray(inputs["W_out"], np.float64)       # (L,d,2d)
    b_out = np.asarray(inputs["b_out"], np.float64)       # (L,2d)
    ln_w = np.asarray(inputs["ln_w"], np.float64)         # (L,d)
    ln_b = np.asarray(inputs["ln_b"], np.float64)
    W_enc = np.asarray(inputs["W_enc"], np.float64)       # (2,d)
    b_enc = np.asarray(inputs["b_enc"], np.float64)       # (d,)
    W_dec = np.asarray(inputs["W_dec"], np.float64)       # (d,1)
    b_dec = np.asarray(inputs["b_dec"], np.float64)       # (1,)
    x = np.asarray(inputs["x"], np.float64)               # (B,H,W,1)
    grid = np.asarray(inputs["grid"], np.float64)

    L = log_dt.shape[0] if n_layers is None else n_layers
    D = log_dt.shape[2] if d_model is None else d_model
    log_dt = log_dt[:L, :, :D]
    logA_re = logA_re[:L, :, :D]
    A_im = A_im[:L, :, :D]
    C_re = C_re[:L, :, :D]
    C_im = C_im[:L, :, :D]
    Dskip = Dskip[:L, :D]
    d_full = W_out.shape[1]
    Wa = W_out[:L, :D, :D]
    Wg = W_out[:L, :D, d_full:d_full + D]
    W_out2 = np.concatenate([Wa, Wg], axis=2)             # (L, D, 2D)
    b_out2 = np.concatenate([b_out[:L, :D], b_out[:L, d_full:d_full + D]], axis=1)
    ln_w = ln_w[:L, :D]
    ln_b = ln_b[:L, :D]
    W_enc = W_enc[:, :D]
    b_enc = b_enc[:D]
    W_dec = W_dec[:D]

    # ---- S4D kernels ----
    dt = np.exp(log_dt)[..., None]                        # (L,2,D,1)
    A = -np.exp(logA_re) + 1j * A_im                      # (L,2,D,N)
    C = C_re + 1j * C_im
    dtA = dt * A
    CB = C * (np.exp(dtA) - 1.0) / A
    t = np.arange(H, dtype=np.float64)
    pows = np.exp(dtA[..., None] * t)                     # (L,2,D,N,H)
    K = 2.0 * np.real(np.einsum("lxdn,lxdnt->lxdt", CB, pows))  # (L,2,D,H)
    kh = K[:, 0]                                          # (L,D,H)
    kw = K[:, 1]                                          # (L,D,W)

    # transposed lower-triangular Toeplitz: ThT[l,d,i,p] = kh[l,d,p-i], p>=i
    idx = np.arange(H)[None, :] - np.arange(H)[:, None]   # (i,p) = p-i
    mask = idx >= 0
    idxc = np.clip(idx, 0, H - 1)
    ThT = np.where(mask, kh[:, :, idxc], 0.0)             # (L,D,128,128)
    TwT = np.where(mask, kw[:, :, idxc], 0.0)

    def pack4(T):
        # (L, D, 128, 128) -> (L, D//4, 128, 512): 4 channels side by side
        Lc, Dc = T.shape[0], T.shape[1]
        return np.ascontiguousarray(
            T.reshape(Lc, Dc // 4, 4, 128, 128)
             .transpose(0, 1, 3, 2, 4)
             .reshape(Lc, Dc // 4, 128, 512)
             .astype(np.float32).astype(bf16))

    flags = dict(
        use_ln_affine=not (np.all(ln_w == 1.0) and np.all(ln_b == 0.0)),
        use_b_out=not np.all(b_out2 == 0.0),
        n_layers=L,
        d_model=D,
        b_dec=float(b_dec[0]),
    )

    # per-channel scaled identities for the D-skip matmul, packed like tht4
    sid = np.zeros((L, D, 128, 128), np.float64)
    ar = np.arange(128)
    sid[:, :, ar, ar] = Dskip[:, :, None]

    flags["wsum"] = float(np.sum(W_dec))

    NK = max(1, D // 128)
    common = dict(
        tht4=pack4(ThT),                                  # (L,D/4,128,512)
        twt4=pack4(TwT),
        sid4=pack4(sid),
        wdec_rep=np.tile(W_dec.astype(np.float32).reshape(1, D), (128, 1)).astype(bf16),
        ident=np.eye(128, dtype=np.float32).astype(bf16),
        wout=np.ascontiguousarray(
            W_out2.reshape(L, NK, min(D, 128), 2 * D).astype(np.float32).astype(bf16)),
    )
    if flags["use_ln_affine"]:
        common["lnw_rep"] = np.tile(ln_w.astype(np.float32)[:, None, :], (1, 128, 1)).astype(bf16)
        common["lnb_rep"] = np.tile(ln_b.astype(np.float32)[:, None, :], (1, 128, 1)).astype(bf16)
    if flags["use_b_out"]:
        common["bout_rep"] = np.tile(b_out2.astype(np.float32)[:, None, :], (1, 128, 1))

    # host-side encoder: h0[b, h, w, d] = x*We0 + grid*We1 + b_enc
    xg = np.stack([x[..., 0], grid[..., 0]], axis=-1)     # (B,H,W,2)
    h0 = xg @ W_enc + b_enc                               # (B,H,W,D) f64
    per_batch = []
    for b in range(x.shape[0]):
        per_batch.append(dict(h0=np.ascontiguousarray(
            h0[b].astype(np.float32).astype(bf16))))      # (H, W, D) = (128, w, d)
    return common, per_batch, flags


def build_program(flags, num_devices=8, gelu_fn=None):
    """Emit the bass program."""
    L = flags["n_layers"]
    D = flags["d_model"]
    NG = D // 4            # channel groups of 4
    NK = max(1, D // 128)  # K tiles in W_out GEMM
    DH = D // 2
    assert D % 8 == 0

    if gelu_fn is None:
        gelu_fn = AF.Gelu_apprx_tanh
    nc = bacc.Bacc("TRN2", target_bir_lowering=False, debug=False,
                   num_devices=num_devices)

    def din(name, shape, dt):
        return nc.dram_tensor(name, shape, dt, kind="ExternalInput").ap()

    h0 = din("h0", [128, W, D], BF)
    tht4 = din("tht4", [L, NG, 128, 512], BF)
    twt4 = din("twt4", [L, NG, 128, 512], BF)
    sid4 = din("sid4", [L, NG, 128, 512], BF)
    wout = din("wout", [L, NK, min(D, 128), 2 * D], BF)
    wdec_rep = din("wdec_rep", [128, D], BF)
    ident = din("ident", [128, 128], BF)
    if flags["use_ln_affine"]:
        lnw_rep = din("lnw_rep", [L, 128, D], BF)
        lnb_rep = din("lnb_rep", [L, 128, D], BF)
    if flags["use_b_out"]:
        bout_rep = din("bout_rep", [L, 128, 2 * D], F32)
    out = nc.dram_tensor("out", [H, W], F32, kind="ExternalOutput").ap()

    from contextlib import ExitStack
    with tile.TileContext(nc) as tc, ExitStack() as ctx:
        state = ctx.enter_context(tc.tile_pool(name="state", bufs=1))
        consts = ctx.enter_context(tc.tile_pool(name="consts", bufs=1))
        lring = ctx.enter_context(tc.tile_pool(name="lring", bufs=2))
        sring = ctx.enter_context(tc.tile_pool(name="sring", bufs=8))
        wring = ctx.enter_context(tc.tile_pool(name="wring", bufs=8))
        atring = ctx.enter_context(tc.tile_pool(name="atring", bufs=3))
        cring = ctx.enter_context(tc.tile_pool(name="cring", bufs=6))
        stats = ctx.enter_context(tc.tile_pool(name="stats", bufs=1))
        psA = ctx.enter_context(tc.tile_pool(name="psA", bufs=2, space="PSUM"))
        psB = ctx.enter_context(tc.tile_pool(name="psB", bufs=2, space="PSUM"))
        psT = ctx.enter_context(tc.tile_pool(name="psT", bufs=2, space="PSUM"))
        psC = ctx.enter_context(tc.tile_pool(name="psC", bufs=2, space="PSUM"))

        h_sb = state.tile([128, W, D], BF, tag="h")
        G = state.tile([128, DH, W], BF, tag="G")   # [p, ch-in-half, w]
        Xts = [state.tile([128, H * W], BF, tag=f"xt{k}", name=f"xt{k}")
               for k in range(NK)]

        ident_t = consts.tile([128, 128], BF)
        nc.sync.dma_start(ident_t[:], ident[:])
        wdec_t = consts.tile([128, D], BF)
        nc.sync.dma_start(wdec_t[:], wdec_rep[:])
        eps_t = consts.tile([128, 1], F32)
        nc.vector.memset(eps_t[:], 1e-5)

        ssum = stats.tile([128, W], F32, tag="ssum")
        sqs = stats.tile([128, W], F32, tag="sqs")
        mu = stats.tile([128, W], F32, tag="mu")
        var = stats.tile([128, W], F32, tag="var")
        std = stats.tile([128, W], F32, tag="std")
        rr = stats.tile([128, W], F32, tag="rr")
        nmr = stats.tile([128, W], F32, tag="nmr")
        dec_sb = stats.tile([128, W], F32, tag="dec")

        # load encoder output straight into the state
        nc.sync.dma_start(h_sb[:], h0[:])

        for l in range(L):
            wout_ts = []
            for k in range(NK):
                wt = lring.tile([min(D, 128), 2 * D], BF, tag="woutw")
                nc.sync.dma_start(wt[:], wout[l, k])
                wout_ts.append(wt)
            if flags["use_ln_affine"]:
                lnw_t = lring.tile([128, D], BF, tag="lnw")
                nc.sync.dma_start(lnw_t[:], lnw_rep[l])
                lnb_t = lring.tile([128, D], BF, tag="lnb")
                nc.sync.dma_start(lnb_t[:], lnb_rep[l])
            if flags["use_b_out"]:
                bout_t = lring.tile([128, 2 * D], F32, tag="bout")
                nc.sync.dma_start(bout_t[:], bout_rep[l])

            # ---------------- phase 1: conv + transpose ----------------
            for hf in range(2):
                ngh = NG // 2
                for g in range(ngh):
                    gd = hf * ngh + g
                    d0 = gd * 4
                    tht_t = wring.tile([128, 512], BF, tag="tht")
                    nc.sync.dma_start(tht_t[:], tht4[l, gd])
                    twt_t = wring.tile([128, 512], BF, tag="twt")
                    nc.sync.dma_start(twt_t[:], twt4[l, gd])
                    ps1 = psA.tile([128, 512], F32, tag="psA")
                    for j in range(4):
                        nc.tensor.matmul(ps1[:, j * 128:(j + 1) * 128],
                                         h_sb[:, :, d0 + j], tht_t[:, j * 128:(j + 1) * 128],
                                         start=True, stop=True)
                    sid_t = sring.tile([128, 512], BF, tag="sid")
                    nc.sync.dma_start(sid_t[:], sid4[l, gd])
                    At4 = atring.tile([128, 512], BF, tag="at")
                    if g % 2 == 0:
                        nc.vector.tensor_copy(At4[:], ps1[:])
                    else:
                        nc.scalar.copy(At4[:], ps1[:])
                    ps2 = psB.tile([128, 512], F32, tag="psB")
                    for j in range(4):
                        nc.tensor.matmul(ps2[:, j * 128:(j + 1) * 128],
                                         At4[:, j * 128:(j + 1) * 128],
                                         twt_t[:, j * 128:(j + 1) * 128],
                                         start=True, stop=False)
                        nc.tensor.matmul(ps2[:, j * 128:(j + 1) * 128],
                                         sid_t[:, j * 128:(j + 1) * 128],
                                         h_sb[:, :, d0 + j],
                                         start=False, stop=True)
                    # gelu -> G (dense write, [p, (c w)])
                    gg = g * 4
                    gout = G[:, gg:gg + 4, :].rearrange("p c w -> p (c w)")
                    nc.scalar.activation(gout, ps2[:], gelu_fn)

                # ---- stage B: transpose this half into Xt ----
                for wb in range(0, W, 8):
                    pst = psT.tile([128, 1024], BF, tag="psT")
                    for t in range(8):
                        nc.tensor.transpose(pst[:, t * 128:(t + 1) * 128],
                                            G[:, :, wb + t], ident_t[:])
                    dst = Xts[hf][:, wb * 128:(wb + 8) * 128]
                    if (wb // 8) % 2 == 0:
                        nc.vector.tensor_copy(dst, pst[:])
                    else:
                        nc.scalar.copy(dst, pst[:])

            # ---------------- phase 2: GEMM + GLU + residual + LN ----------
            # last layer: LN output feeds only the decoder; fold both:
            # out = rr*(sum(p*wdec) - mu*sum(wdec)) + b_dec
            fold_dec = (l == L - 1) and not flags["use_ln_affine"]
            SB = 64  # stats batch
            for w0 in range(W):
                psW = psC.tile([128, 2 * D], F32, tag="psC")
                for k in range(NK):
                    nc.tensor.matmul(psW[:], Xts[k][:, w0 * 128:(w0 + 1) * 128],
                                     wout_ts[k][:], start=(k == 0),
                                     stop=(k == NK - 1))
                if flags["use_b_out"]:
                    nc.vector.tensor_tensor(psW[:], psW[:], bout_t[:], op=OP.add)
                sig = cring.tile([128, D], BF, tag="sig")
                nc.scalar.activation(sig[:], psW[:, D:2 * D], AF.Sigmoid)
                glu = cring.tile([128, D], BF, tag="glu")
                h_slot = h_sb[:, w0, :]
                if l == 0:
                    # encoder h is not zero-mean: accumulate sum on residual
                    nc.vector.scalar_tensor_tensor(
                        out=glu[:], in0=psW[:, 0:D], scalar=1.0, in1=sig[:],
                        op0=OP.mult, op1=OP.mult)
                    nc.vector.scalar_tensor_tensor(
                        out=h_slot, in0=glu[:], scalar=1.0, in1=h_slot,
                        op0=OP.mult, op1=OP.add, accum_out=ssum[:, w0:w0 + 1])
                else:
                    # post-LN h sums to zero over d: sum(p) == sum(glu)
                    nc.vector.scalar_tensor_tensor(
                        out=glu[:], in0=psW[:, 0:D], scalar=1.0, in1=sig[:],
                        op0=OP.mult, op1=OP.mult, accum_out=ssum[:, w0:w0 + 1])
                    nc.vector.tensor_tensor(h_slot, glu[:], h_slot, op=OP.add)
                scr = cring.tile([128, D], BF, tag="scr")
                nc.vector.scalar_tensor_tensor(
                    out=scr[:], in0=h_slot, scalar=1.0, in1=h_slot,
                    op0=OP.mult, op1=OP.mult, accum_out=sqs[:, w0:w0 + 1])
                if fold_dec:
                    scr2 = cring.tile([128, D], BF, tag="scr2")
                    nc.vector.scalar_tensor_tensor(
                        out=scr2[:], in0=h_slot, scalar=1.0, in1=wdec_t[:],
                        op0=OP.mult, op1=OP.mult,
                        accum_out=dec_sb[:, w0:w0 + 1])
                if w0 % SB == SB - 1:
                    b0 = w0 - SB + 1
                    blk = slice(b0, w0 + 1)
                    nc.vector.tensor_scalar(out=mu[:, blk], in0=ssum[:, blk],
                                            scalar1=1.0 / D, scalar2=None,
                                            op0=OP.mult)
                    nc.vector.tensor_tensor(var[:, blk], mu[:, blk], mu[:, blk],
                                            op=OP.mult)
                    nc.vector.scalar_tensor_tensor(
                        out=var[:, blk], in0=sqs[:, blk], scalar=1.0 / D,
                        in1=var[:, blk], op0=OP.mult, op1=OP.subtract)
                    nc.scalar.activation(std[:, blk], var[:, blk], AF.Sqrt,
                                         bias=eps_t[:, 0:1])
                    nc.vector.reciprocal(rr[:, blk], std[:, blk])
                    if fold_dec:
                        # dec = (dec_raw - mu*wsum) * rr
                        nc.vector.scalar_tensor_tensor(
                            out=dec_sb[:, blk], in0=mu[:, blk],
                            scalar=-float(flags["wsum"]),
                            in1=dec_sb[:, blk], op0=OP.mult, op1=OP.add)
                        nc.vector.tensor_tensor(dec_sb[:, blk], dec_sb[:, blk],
                                                rr[:, blk], op=OP.mult)
                        continue
                    nc.vector.scalar_tensor_tensor(
                        out=nmr[:, blk], in0=mu[:, blk], scalar=-1.0,
                        in1=rr[:, blk], op0=OP.mult, op1=OP.mult)
                    for wv in range(b0, w0 + 1):
                        hv = h_sb[:, wv, :]
                        # first block: ACT only (DVE still busy with the
                        # remaining w0); tail block: split DVE/ACT to halve
                        # the serial tail before the next layer's conv.
                        if wv >= W - SB and wv % 2 == 0:
                            nc.vector.scalar_tensor_tensor(
                                out=hv, in0=hv, scalar=rr[:, wv:wv + 1],
                                op0=OP.mult,
                                in1=nmr[:, wv:wv + 1].broadcast_to((128, D)),
                                op1=OP.add)
                        else:
                            nc.scalar.activation(
                                hv, hv, AF.Identity,
                                bias=nmr[:, wv:wv + 1], scale=rr[:, wv:wv + 1])
                        if flags["use_ln_affine"]:
                            nc.vector.tensor_tensor(hv, hv, lnw_t[:], op=OP.mult)
                            nc.vector.tensor_tensor(hv, hv, lnb_t[:], op=OP.add)

        # ---------------- decoder (only if not folded into last layer) ------
        if flags["use_ln_affine"]:
            for w0 in range(W):
                scr = cring.tile([128, D], BF, tag="scr")
                nc.vector.scalar_tensor_tensor(
                    out=scr[:], in0=h_sb[:, w0, :], scalar=1.0, in1=wdec_t[:],
                    op0=OP.mult, op1=OP.mult, accum_out=dec_sb[:, w0:w0 + 1])
        if flags["b_dec"] != 0.0:
            nc.vector.tensor_scalar(out=dec_sb[:], in0=dec_sb[:],
                                    scalar1=float(flags["b_dec"]), scalar2=None,
                                    op0=OP.add)
        nc.sync.dma_start(out[:], dec_sb[:])

    nc.compile()
    return nc


# ---------------------------------------------------------------------------
# Self-contained entry point: full inputs in, full output out.
# ---------------------------------------------------------------------------

_PROGRAM_CACHE = {}


def _get_program(flags):
    key = (flags["n_layers"], flags["d_model"], flags["use_ln_affine"],
           flags["use_b_out"], flags["b_dec"], flags.get("wsum"))
    if key not in _PROGRAM_CACHE:
        _PROGRAM_CACHE[key] = build_program(flags, num_devices=8)
    return _PROGRAM_CACHE[key]


def kernel(**inputs):
    import os
    from concourse.bass_utils import run_bass_kernel_spmd

    common, per_batch, flags = host_prep(inputs)
    nc = _get_program(flags)

    B = len(per_batch)
    in_maps = []
    for c in range(8):
        m = dict(common)
        m.update(per_batch[c % B])
        in_maps.append(m)

    trace = bool(os.environ.get("S4ND_TRACE"))
    res = run_bass_kernel_spmd(nc, in_maps, core_ids=list(range(8)), trace=trace)
    if trace and res.exec_time_ns is not None:
        print(f"HW exec time: {res.exec_time_ns} ns")
        kernel.last_exec_time_ns = res.exec_time_ns
        kernel.last_results = res

    out = np.stack([res.results[b]["out"] for b in range(B)], axis=0)[..., None]
    return out.astype(np.float32)


# revision 23
# speedup vs baseline: 1.0581x; 1.0286x over previous
"""S4ND Darcy-flow Bass kernel v2: builder + host-side preparation.

Design (per core = one batch element, batch-parallel over 4 cores, cores
4..7 duplicate work and are ignored at gather time):

  state h_sb: SBUF bf16 [128p=h, (w, d)], d innermost.
  Encoder precomputed on host (h0 DMA'd straight into h_sb).
  Per layer:
    phase 1 (conv), channels in groups of 4:
      MM1 x4:  ps1[:, j*128] = U_d^T @ ThT_d        (lhsT = U_d strided slice)
      copy:    ps1 -> At4 bf16 (DVE/ACT alternating, [128,512])
      sident:  DVE tensor_scalar builds D_d * I from identity
      MM2 x4:  ps2[:, j*128] = At^T @ TwT_d  (+ D_d*I^T @ U_d accumulated)
      gelu:    one ACT op [128,512] -> G[p, ch, w] (dense write)
      stage B: per 8 w0: 8 PE transposes into one PSUM bank [128,1024] bf16,
               one DVE/ACT copy out to Xt (channel-major).
    phase 2 (per w0): GEMM psW = Xt0@W0 + Xt1@W1; ACT sigmoid; DVE glu
      (+ssum accum via sum(glu)=sum(p), valid post-LN layers); DVE residual
      add in place into h_sb; DVE sumsq accum; batched stats; normalize
      in place (ACT Identity with per-partition scale/bias, DVE share).
  Decoder: DVE stt dot-products per w slice -> out (h, w) f32.

Host precomputes (numpy, float64): S4D kernels kh/kw, transposed Toeplitz
matrices ThT/TwT packed 4 channels per tile row for dense DMA.
"""

import numpy as np
import ml_dtypes

import concourse.bacc as bacc
import concourse.mybir as mybir
import concourse.tile as tile

bf16 = ml_dtypes.bfloat16
AF = mybir.ActivationFunctionType
OP = mybir.AluOpType
F32 = mybir.dt.float32
BF = mybir.dt.bfloat16

H = 128
W = 128

# normalize assignment pattern: of every 4 w0, this many normalize on DVE
# (via stt with broadcast in1); the rest on ACT (Identity w/ scale+bias).
NORM_DVE_FRACTION = 2


def host_prep(inputs, n_layers=None, d_model=None):
    """Compute device-side constant tensors from the full model inputs."""
    log_dt = np.asarray(inputs["log_dt"], np.float64)     # (L,2,d)
    logA_re = np.asarray(inputs["logA_re"], np.float64)   # (L,2,d,N)
    A_im = np.asarray(inputs["A_im"], np.float64)
    C_re = np.asarray(inputs["C_re"], np.float64)
    C_im = np.asarray(inputs["C_im"], np.float64)
    Dskip = np.asarray(inputs["Dskip"], np.float64)       # (L,d)
    W_out = np.asarray(inputs["W_out"], np.float64)       # (L,d,2d)
    b_out = np.asarray(inputs["b_out"], np.float64)       # (L,2d)
    ln_w = np.asarray(inputs["ln_w"], np.float64)         # (L,d)
    ln_b = np.asarray(inputs["ln_b"], np.float64)
    W_enc = np.asarray(inputs["W_enc"], np.float64)       # (2,d)
    b_enc = np.asarray(inputs["b_enc"], np.float64)       # (d,)
    W_dec = np.asarray(inputs["W_dec"], np.float64)       # (d,1)
    b_dec = np.asarray(inputs["b_dec"], np.float64)       # (1,)
    x = np.asarray(inputs["x"], np.float64)               # (B,H,W,1)
    grid = np.asarray(inputs["grid"], np.float64)

    L = log_dt.shape[0] if n_layers is None else n_layers
    D = log_dt.shape[2] if d_model is None else d_model
    log_dt = log_dt[:L, :, :D]
    logA_re = logA_re[:L, :, :D]
    A_im = A_im[:L, :, :D]
    C_re = C_re[:L, :, :D]
    C_im = C_im[:L, :, :D]
    Dskip = Dskip[:L, :D]
    d_full = W_out.shape[1]
    Wa = W_out[:L, :D, :D]
    Wg = W_out[:L, :D, d_full:d_full + D]
    W_out2 = np.concatenate([Wa, Wg], axis=2)             # (L, D, 2D)
    b_out2 = np.concatenate([b_out[:L, :D], b_out[:L, d_full:d_full + D]], axis=1)
    ln_w = ln_w[:L, :D]
    ln_b = ln_b[:L, :D]
    W_enc = W_enc[:, :D]
    b_enc = b_enc[:D]
    W_dec = W_dec[:D]

    # ---- S4D kernels ----
    dt = np.exp(log_dt)[..., None]                        # (L,2,D,1)
    A = -np.exp(logA_re) + 1j * A_im                      # (L,2,D,N)
    C = C_re + 1j * C_im
    dtA = dt * A
    CB = C * (np.exp(dtA) - 1.0) / A
    t = np.arange(H, dtype=np.float64)
    pows = np.exp(dtA[..., None] * t)                     # (L,2,D,N,H)
    K = 2.0 * np.real(np.einsum("lxdn,lxdnt->lxdt", CB, pows))  # (L,2,D,H)
    kh = K[:, 0]                                          # (L,D,H)
    kw = K[:, 1]                                          # (L,D,W)

    # transposed lower-triangular Toeplitz: ThT[l,d,i,p] = kh[l,d,p-i], p>=i
    idx = np.arange(H)[None, :] - np.arange(H)[:, None]   # (i,p) = p-i
    mask = idx >= 0
    idxc = np.clip(idx, 0, H - 1)
    ThT = np.where(mask, kh[:, :, idxc], 0.0)             # (L,D,128,128)
    TwT = np.where(mask, kw[:, :, idxc], 0.0)

    def pack4(T):
        # (L, D, 128, 128) -> (L, D//4, 128, 512): 4 channels side by side
        Lc, Dc = T.shape[0], T.shape[1]
        return np.ascontiguousarray(
            T.reshape(Lc, Dc // 4, 4, 128, 128)
             .transpose(0, 1, 3, 2, 4)
             .reshape(Lc, Dc // 4, 128, 512)
             .astype(np.float32).astype(bf16))

    flags = dict(
        use_ln_affine=not (np.all(ln_w == 1.0) and np.all(ln_b == 0.0)),
        use_b_out=not np.all(b_out2 == 0.0),
        n_layers=L,
        d_model=D,
        b_dec=float(b_dec[0]),
    )

    # per-channel scaled identities for the D-skip matmul, packed like tht4
    sid = np.zeros((L, D, 128, 128), np.float64)
    ar = np.arange(128)
    sid[:, :, ar, ar] = Dskip[:, :, None]

    flags["wsum"] = float(np.sum(W_dec))

    NK = max(1, D // 128)
    common = dict(
        tht4=pack4(ThT),                                  # (L,D/4,128,512)
        twt4=pack4(TwT),
        sid4=pack4(sid),
        wdec_rep=np.tile(W_dec.astype(np.float32).reshape(1, D), (128, 1)).astype(bf16),
        ident=np.eye(128, dtype=np.float32).astype(bf16),
        wout=np.ascontiguousarray(
            W_out2.reshape(L, NK, min(D, 128), 2 * D).astype(np.float32).astype(bf16)),
    )
    if flags["use_ln_affine"]:
        common["lnw_rep"] = np.tile(ln_w.astype(np.float32)[:, None, :], (1, 128, 1)).astype(bf16)
        common["lnb_rep"] = np.tile(ln_b.astype(np.float32)[:, None, :], (1, 128, 1)).astype(bf16)
    if flags["use_b_out"]:
        common["bout_rep"] = np.tile(b_out2.astype(np.float32)[:, None, :], (1, 128, 1))

    # host-side encoder: h0[b, h, w, d] = x*We0 + grid*We1 + b_enc
    xg = np.stack([x[..., 0], grid[..., 0]], axis=-1)     # (B,H,W,2)
    h0 = xg @ W_enc + b_enc                               # (B,H,W,D) f64
    per_batch = []
    for b in range(x.shape[0]):
        per_batch.append(dict(h0=np.ascontiguousarray(
            h0[b].astype(np.float32).astype(bf16))))      # (H, W, D) = (128, w, d)
    return common, per_batch, flags


def build_program(flags, num_devices=8, gelu_fn=None):
    """Emit the bass program."""
    L = flags["n_layers"]
    D = flags["d_model"]
    NG = D // 4            # channel groups of 4
    NK = max(1, D // 128)  # K tiles in W_out GEMM
    DH = D // 2
    assert D % 8 == 0

    if gelu_fn is None:
        gelu_fn = AF.Gelu_apprx_tanh
    nc = bacc.Bacc("TRN2", target_bir_lowering=False, debug=False,
                   num_devices=num_devices)

    def din(name, shape, dt):
        return nc.dram_tensor(name, shape, dt, kind="ExternalInput").ap()

    h0 = din("h0", [128, W, D], BF)
    tht4 = din("tht4", [L, NG, 128, 512], BF)
    twt4 = din("twt4", [L, NG, 128, 512], BF)
    sid4 = din("sid4", [L, NG, 128, 512], BF)
    wout = din("wout", [L, NK, min(D, 128), 2 * D], BF)
    wdec_rep = din("wdec_rep", [128, D], BF)
    ident = din("ident", [128, 128], BF)
    if flags["use_ln_affine"]:
        lnw_rep = din("lnw_rep", [L, 128, D], BF)
        lnb_rep = din("lnb_rep", [L, 128, D], BF)
    if flags["use_b_out"]:
        bout_rep = din("bout_rep", [L, 128, 2 * D], F32)
    out = nc.dram_tensor("out", [H, W], F32, kind="ExternalOutput").ap()

    from contextlib import ExitStack
    with tile.TileContext(nc) as tc, ExitStack() as ctx:
        state = ctx.enter_context(tc.tile_pool(name="state", bufs=1))
        consts = ctx.enter_context(tc.tile_pool(name="consts", bufs=1))
        lring = ctx.enter_context(tc.tile_pool(name="lring", bufs=2))
        sring = ctx.enter_context(tc.tile_pool(name="sring", bufs=8))
        wring = ctx.enter_context(tc.tile_pool(name="wring", bufs=8))
        atring = ctx.enter_context(tc.tile_pool(name="atring", bufs=3))
        cring = ctx.enter_context(tc.tile_pool(name="cring", bufs=6))
        stats = ctx.enter_context(tc.tile_pool(name="stats", bufs=1))
        psA = ctx.enter_context(tc.tile_pool(name="psA", bufs=2, space="PSUM"))
        psB = ctx.enter_context(tc.tile_pool(name="psB", bufs=2, space="PSUM"))
        psT = ctx.enter_context(tc.tile_pool(name="psT", bufs=2, space="PSUM"))
        psC = ctx.enter_context(tc.tile_pool(name="psC", bufs=2, space="PSUM"))

        h_sb = state.tile([128, W, D], BF, tag="h")
        G = state.tile([128, DH, W], BF, tag="G")   # [p, ch-in-half, w]
        Xts = [state.tile([128, H * W], BF, tag=f"xt{k}", name=f"xt{k}")
               for k in range(NK)]

        ident_t = consts.tile([128, 128], BF)
        nc.sync.dma_start(ident_t[:], ident[:])
        wdec_t = consts.tile([128, D], BF)
        nc.sync.dma_start(wdec_t[:], wdec_rep[:])
        eps_t = consts.tile([128, 1], F32)
        nc.vector.memset(eps_t[:], 1e-5)

        ssum = stats.tile([128, W], F32, tag="ssum")
        sqs = stats.tile([128, W], F32, tag="sqs")
        mu = stats.tile([128, W], F32, tag="mu")
        var = stats.tile([128, W], F32, tag="var")
        std = stats.tile([128, W], F32, tag="std")
        rr = stats.tile([128, W], F32, tag="rr")
        nmr = stats.tile([128, W], F32, tag="nmr")
        dec_sb = stats.tile([128, W], F32, tag="dec")

        # load encoder output straight into the state
        nc.sync.dma_start(h_sb[:], h0[:])

        for l in range(L):
            wout_ts = []
            for k in range(NK):
                wt = lring.tile([min(D, 128), 2 * D], BF, tag="woutw")
                nc.sync.dma_start(wt[:], wout[l, k])
                wout_ts.append(wt)
            if flags["use_ln_affine"]:
                lnw_t = lring.tile([128, D], BF, tag="lnw")
                nc.sync.dma_start(lnw_t[:], lnw_rep[l])
                lnb_t = lring.tile([128, D], BF, tag="lnb")
                nc.sync.dma_start(lnb_t[:], lnb_rep[l])
            if flags["use_b_out"]:
                bout_t = lring.tile([128, 2 * D], F32, tag="bout")
                nc.sync.dma_start(bout_t[:], bout_rep[l])

            # ---------------- phase 1: conv + transpose ----------------
            for hf in range(2):
                ngh = NG // 2
                for g in range(ngh):
                    gd = hf * ngh + g
                    d0 = gd * 4
                    tht_t = wring.tile([128, 512], BF, tag="tht")
                    nc.sync.dma_start(tht_t[:], tht4[l, gd])
                    twt_t = wring.tile([128, 512], BF, tag="twt")
                    nc.sync.dma_start(twt_t[:], twt4[l, gd])
                    ps1 = psA.tile([128, 512], F32, tag="psA")
                    for j in range(4):
                        nc.tensor.matmul(ps1[:, j * 128:(j + 1) * 128],
                                         h_sb[:, :, d0 + j], tht_t[:, j * 128:(j + 1) * 128],
                                         start=True, stop=True)
                    sid_t = sring.tile([128, 512], BF, tag="sid")
                    nc.sync.dma_start(sid_t[:], sid4[l, gd])
                    At4 = atring.tile([128, 512], BF, tag="at")
                    if g % 2 == 0:
                        nc.vector.tensor_copy(At4[:], ps1[:])
                    else:
                        nc.scalar.copy(At4[:], ps1[:])
                    ps2 = psB.tile([128, 512], F32, tag="psB")
                    for j in range(4):
                        nc.tensor.matmul(ps2[:, j * 128:(j + 1) * 128],
                                         At4[:, j * 128:(j + 1) * 128],
                                         twt_t[:, j * 128:(j + 1) * 128],
                                         start=True, stop=False)
                        nc.tensor.matmul(ps2[:, j * 128:(j + 1) * 128],
                                         sid_t[:, j * 128:(j + 1) * 128],
                                         h_sb[:, :, d0 + j],
                                         start=False, stop=True)
                    # gelu -> G (dense write, [p, (c w)])
                    gg = g * 4
                    gout = G[:, gg:gg + 4, :].rearrange("p c w -> p (c w)")
                    nc.scalar.activation(gout, ps2[:], gelu_fn)

                # ---- stage B: transpose this half into Xt ----
                for wb in range(0, W, 8):
                    pst = psT.tile([128, 1024], BF, tag="psT")
                    for t in range(8):
                        nc.tensor.transpose(pst[:, t * 128:(t + 1) * 128],
                                            G[:, :, wb + t], ident_t[:])
                    dst = Xts[hf][:, wb * 128:(wb + 8) * 128]
                    if (wb // 8) % 2 == 0:
                        nc.vector.tensor_copy(dst, pst[:])
                    else:
                        nc.scalar.copy(dst, pst[:])

            # ---------------- phase 2: GEMM + GLU + residual + LN ----------
            # last layer: LN output feeds only the decoder; fold both:
            # out = rr*(sum(p*wdec) - mu*sum(wdec)) + b_dec
            fold_dec = (l == L - 1) and not flags["use_ln_affine"]
            SB = 64  # stats batch
            for w0 in range(W):
                psW = psC.tile([128, 2 * D], F32, tag="psC")
                for k in range(NK):
                    nc.tensor.matmul(psW[:], Xts[k][:, w0 * 128:(w0 + 1) * 128],
                                     wout_ts[k][:], start=(k == 0),
                                     stop=(k == NK - 1))
                if flags["use_b_out"]:
                    nc.vector.tensor_tensor(psW[:], psW[:], bout_t[:], op=OP.add)
                sig = cring.tile([128, D], BF, tag="sig")
                nc.scalar.activation(sig[:], psW[:, D:2 * D], AF.Sigmoid)
                glu = cring.tile([128, D], BF, tag="glu")
                h_slot = h_sb[:, w0, :]
                if l == 0:
                    # encoder h is not zero-mean: accumulate sum on residual
                    nc.vector.scalar_tensor_tensor(
                        out=glu[:], in0=psW[:, 0:D], scalar=1.0, in1=sig[:],
                        op0=OP.mult, op1=OP.mult)
                    nc.vector.scalar_tensor_tensor(
                        out=h_slot, in0=glu[:], scalar=1.0, in1=h_slot,
                        op0=OP.mult, op1=OP.add, accum_out=ssum[:, w0:w0 + 1])
                else:
                    # post-LN h sums to zero over d: sum(p) == sum(glu)
                    nc.vector.scalar_tensor_tensor(
                        out=glu[:], in0=psW[:, 0:D], scalar=1.0, in1=sig[:],
                        op0=OP.mult, op1=OP.mult, accum_out=ssum[:, w0:w0 + 1])
                    nc.vector.tensor_tensor(h_slot, glu[:], h_slot, op=OP.add)
                scr = cring.tile([128, D], BF, tag="scr")
                nc.vector.scalar_tensor_tensor(
                    out=scr[:], in0=h_slot, scalar=1.0, in1=h_slot,
                    op0=OP.mult, op1=OP.mult, accum_out=sqs[:, w0:w0 + 1])
                if fold_dec:
                    scr2 = cring.tile([128, D], BF, tag="scr2")
                    nc.vector.scalar_tensor_tensor(
                        out=scr2[:], in0=h_slot, scalar=1.0, in1=wdec_t[:],
                        op0=OP.mult, op1=OP.mult,
                        accum_out=dec_sb[:, w0:w0 + 1])
                if w0 % SB == SB - 1:
                    b0 = w0 - SB + 1
                    blk = slice(b0, w0 + 1)
                    nc.vector.tensor_scalar(out=mu[:, blk], in0=ssum[:, blk],
                                            scalar1=1.0 / D, scalar2=None,
                                            op0=OP.mult)
                    nc.vector.tensor_tensor(var[:, blk], mu[:, blk], mu[:, blk],
                                            op=OP.mult)
                    nc.vector.scalar_tensor_tensor(
                        out=var[:, blk], in0=sqs[:, blk], scalar=1.0 / D,
                        in1=var[:, blk], op0=OP.mult, op1=OP.subtract)
                    nc.scalar.activation(std[:, blk], var[:, blk], AF.Sqrt,
                                         bias=eps_t[:, 0:1])
                    nc.vector.reciprocal(rr[:, blk], std[:, blk])
                    if fold_dec:
                        # dec = (dec_raw - mu*wsum) * rr
                        nc.vector.scalar_tensor_tensor(
                            out=dec_sb[:, blk], in0=mu[:, blk],
                            scalar=-float(flags["wsum"]),
                            in1=dec_sb[:, blk], op0=OP.mult, op1=OP.add)
                        nc.vector.tensor_tensor(dec_sb[:, blk], dec_sb[:, blk],
                                                rr[:, blk], op=OP.mult)
                        continue
                    nc.vector.scalar_tensor_tensor(
                        out=nmr[:, blk], in0=mu[:, blk], scalar=-1.0,
                        in1=rr[:, blk], op0=OP.mult, op1=OP.mult)
                    for wv in range(b0, w0 + 1):
                        hv = h_sb[:, wv, :]
                        if wv % 4 < NORM_DVE_FRACTION:
                            nc.vector.scalar_tensor_tensor(
                                out=hv, in0=hv, scalar=rr[:, wv:wv + 1],
                                op0=OP.mult,
                                in1=nmr[:, wv:wv + 1].broadcast_to((128, D)),
                                op1=OP.add)
                        else:
                            nc.scalar.activation(
                                hv, hv, AF.Identity,
                                bias=nmr[:, wv:wv + 1], scale=rr[:, wv:wv + 1])
                        if flags["use_ln_affine"]:
                            nc.vector.tensor_tensor(hv, hv, lnw_t[:], op=OP.mult)
                            nc.vector.tensor_tensor(hv, hv, lnb_t[:], op=OP.add)

        # ---------------- decoder (only if not folded into last layer) ------
        if flags["use_ln_affine"]:
            for w0 in range(W):
                scr = cring.tile([128, D], BF, tag="scr")
                nc.vector.scalar_tensor_tensor(
                    out=scr[:], in0=h_sb[:, w0, :], scalar=1.0, in1=wdec_t[:],
                    op0=OP.mult, op1=OP.mult, accum_out=dec_sb[:, w0:w0 + 1])
        if flags["b_dec"] != 0.0:
            nc.vector.tensor_scalar(out=dec_sb[:], in0=dec_sb[:],
                                    scalar1=float(flags["b_dec"]), scalar2=None,
                                    op0=OP.add)
        nc.sync.dma_start(out[:], dec_sb[:])

    nc.compile()
    return nc


# ---------------------------------------------------------------------------
# Self-contained entry point: full inputs in, full output out.
# ---------------------------------------------------------------------------

_PROGRAM_CACHE = {}


def _get_program(flags):
    key = (flags["n_layers"], flags["d_model"], flags["use_ln_affine"],
           flags["use_b_out"], flags["b_dec"], flags.get("wsum"))
    if key not in _PROGRAM_CACHE:
        _PROGRAM_CACHE[key] = build_program(flags, num_devices=8)
    return _PROGRAM_CACHE[key]


def kernel(**inputs):
    import os
    from concourse.bass_utils import run_bass_kernel_spmd

    common, per_batch, flags = host_prep(inputs)
    nc = _get_program(flags)

    B = len(per_batch)
    in_maps = []
    for c in range(8):
        m = dict(common)
        m.update(per_batch[c % B])
        in_maps.append(m)

    trace = bool(os.environ.get("S4ND_TRACE"))
    res = run_bass_kernel_spmd(nc, in_maps, core_ids=list(range(8)), trace=trace)
    if trace and res.exec_time_ns is not None:
        print(f"HW exec time: {res.exec_time_ns} ns")
        kernel.last_exec_time_ns = res.exec_time_ns
        kernel.last_results = res

    out = np.stack([res.results[b]["out"] for b in range(B)], axis=0)[..., None]
    return out.astype(np.float32)
